# revision 10
# baseline (speedup 1.0000x reference)
"""LDA loss (inter/intra hinge) on 8 Trainium2 NeuronCores.

Strategy (data-parallel over B; G=B/16 centers; all-bf16 data path):

  Host staging: cast path_fea to bf16, rearrange each core's shard to
    p-major [128, b, d] so the device load is fully contiguous.

  Launch 1 (per core, 16384 samples):
    - centers via PE selector matmul ([8, 2048] psum per 16-tile chunk)
    - ACT casts centers to bf16 (export + reuse)
    - DMA replicates center rows across partition groups (16x)
    - DVE diff = x - c, ACT square, DVE segmented reduce -> d2 [128,128]
    - tiny f32 hinge tail -> ipart [128, 1]

  Host: gather centers (bf16), build per-core rotated center panels.

  Launch 2 (per core, cyclic-half of the GxG pairwise matrix):
    Uniform triangle: 16 row-chunks of 512; core c owns chunks c and c+8.
    Each row-chunk processes 9 column blocks (its own + next 8 mod 16)
    from a rotated+extended center panel [128, 8704]:
      psum = -2*C_loc^T C_ext + (ones^T [hi;lo]) (K=2 matmul adds ||c_j||^2)
      ACT: t = sqrt(psum + ||c_i||^2 + eps)  (bias per partition)
      DVE: w = min(t,1)-1 ; sum w^2 via scalar_tensor_tensor accum
    Separate accumulators for diag block (self pairs + double-counted),
    middle blocks (counted once), and far block (double-counted).
  Host: inter = (S_mid + (S_diag - S_self)/2 + S_far/2) / n_pairs.

Exact-zero property: every pair with d >= 1 yields min(t,1)-1 = 0
exactly, so the all-zero inter loss is reproduced up to the tiny
host-side self-pair correction (~1e-7 absolute).
"""
import sys

if "/opt/trn_rl_repo" not in sys.path:
    sys.path.insert(0, "/opt/trn_rl_repo")

import numpy as np
import ml_dtypes

import concourse.bacc as bacc
import concourse.tile as tile
from concourse import mybir
from concourse.bass_utils import run_bass_kernel_spmd

N_CORES = 8
B, D, P = 131072, 128, 16
G = B // P                 # 8192 centers
GL = G // N_CORES          # 1024 local centers (rows) per core
SL = B // N_CORES          # 16384 local samples
NT = SL // 128             # 128 sample tiles / core
CW = 512                   # row-chunk width (16 chunks globally)
EXT = 17 * CW              # 8704 extended column panel
EPS = 1e-3
MI = 0.1

F32 = mybir.dt.float32
BF16 = mybir.dt.bfloat16
AF = mybir.ActivationFunctionType
OP = mybir.AluOpType
AX = mybir.AxisListType

_cache = {}
_last_traces = {}


def _build_launch1():
    nc = bacc.Bacc("TRN2", target_bir_lowering=False, debug=False,
                   num_devices=N_CORES)
    xp = nc.dram_tensor("xp", [128, SL], BF16, kind="ExternalInput").ap()
    sel = nc.dram_tensor("sel", [128, 8], BF16, kind="ExternalInput").ap()
    cpack = nc.dram_tensor("cpack", [8, SL], BF16, kind="ExternalOutput").ap()
    ipart = nc.dram_tensor("ipart", [128, 1], F32, kind="ExternalOutput").ap()

    with tile.TileContext(nc) as tc:
        with (
            tc.tile_pool(name="persist", bufs=1) as pp,
            tc.tile_pool(name="work", bufs=3) as wp,
            tc.tile_pool(name="ps1", bufs=2, space="PSUM") as psp,
        ):
            t_xp = pp.tile([128, SL], BF16, tag="xp")
            for k in range(4):
                nc.sync.dma_start(t_xp[:, 4096 * k:4096 * (k + 1)],
                                  xp[:, 4096 * k:4096 * (k + 1)])
            t_sel = pp.tile([128, 8], BF16, tag="sel")
            nc.sync.dma_start(t_sel[:], sel[:])
            t_ct = pp.tile([128, SL], BF16, tag="ct")
            t_d2 = pp.tile([128, NT], F32, tag="d2")
            t_cseg = pp.tile([8, SL], BF16, tag="cseg")

            # centers: selector matmul per 2048-col chunk, cast to bf16
            for k in range(8):
                sl2 = slice(2048 * k, 2048 * (k + 1))
                cps = psp.tile([128, 2048], F32, tag="cps")
                for j in range(4):
                    nc.tensor.matmul(
                        cps[:8, 512 * j:512 * (j + 1)], t_sel[:, :],
                        t_xp[:, 2048 * k + 512 * j:2048 * k + 512 * (j + 1)],
                        start=True, stop=True)
                nc.scalar.copy(t_cseg[:, sl2], cps[:8, :])
            nc.sync.dma_start(cpack[:, :], t_cseg[:, :])
            # replicate center rows across their 16 sample partitions
            # (2 half-row groups of 16 strided DMAs each)
            for h in range(2):
                slh = slice(8192 * h, 8192 * (h + 1))
                for j in range(16):
                    nc.sync.dma_start(t_ct[j::16, slh], t_cseg[:, slh])

            for k in range(8):
                sl2 = slice(2048 * k, 2048 * (k + 1))
                diff = wp.tile([128, 2048], BF16, tag="diff")
                nc.vector.tensor_tensor(diff[:], t_xp[:, sl2], t_ct[:, sl2],
                                        op=OP.subtract)
                dsq = wp.tile([128, 2048], F32, tag="dsq")
                nc.scalar.activation(dsq[:], diff[:], AF.Square)
                nc.vector.tensor_reduce(
                    t_d2[:, 16 * k:16 * (k + 1)],
                    dsq[:].rearrange("p (b d) -> p b d", d=128),
                    axis=AX.X, op=OP.add)

            t_di = pp.tile([128, NT], F32, tag="di")
            nc.scalar.activation(t_di[:], t_d2[:], AF.Sqrt)
            t_w = pp.tile([128, NT], F32, tag="w")
            nc.vector.tensor_scalar(t_w[:], t_di[:], MI, 0.0,
                                    op0=OP.subtract, op1=OP.max)
            t_w2 = pp.tile([128, NT], F32, tag="w2")
            t_acc = pp.tile([128, 1], F32, tag="acc")
            nc.vector.scalar_tensor_tensor(t_w2[:], t_w[:], 0.0, t_w[:],
                                           op0=OP.bypass, op1=OP.mult,
                                           accum_out=t_acc[:])
            nc.sync.dma_start(ipart[:], t_acc[:])
    nc.compile()
    return nc


def _build_launch2():
    nc = bacc.Bacc("TRN2", target_bir_lowering=False, debug=False,
                   num_devices=N_CORES)
    ctr = nc.dram_tensor("ctr", [128, EXT], BF16, kind="ExternalInput").ap()
    lh = nc.dram_tensor("lh", [128, GL], BF16, kind="ExternalInput").ap()
    sqrow = nc.dram_tensor("sqrow", [2, EXT], BF16, kind="ExternalInput").ap()
    sqbias = nc.dram_tensor("sqbias", [128, 8], F32, kind="ExternalInput").ap()
    sqbias2 = nc.dram_tensor("sqbias2", [128, 8], F32,
                             kind="ExternalInput").ap()
    ones1 = nc.dram_tensor("ones1", [2, 128], BF16, kind="ExternalInput").ap()
    accs = nc.dram_tensor("accs", [128, 32], F32, kind="ExternalOutput").ap()

    with tile.TileContext(nc) as tc:
        with (
            tc.tile_pool(name="persist", bufs=1) as pp,
            tc.tile_pool(name="work", bufs=3) as wp,
            tc.tile_pool(name="ps", bufs=2, space="PSUM") as psp,
        ):
            t_ctr = pp.tile([128, EXT], BF16, tag="ctr")
            for k in range(4):
                nc.sync.dma_start(t_ctr[:, 2048 * k:2048 * (k + 1)],
                                  ctr[:, 2048 * k:2048 * (k + 1)])
            nc.sync.dma_start(t_ctr[:, 8192:EXT], ctr[:, 8192:EXT])
            t_lh = pp.tile([128, GL], BF16, tag="lh")
            nc.sync.dma_start(t_lh[:], lh[:])
            t_sq = pp.tile([2, EXT], BF16, tag="sq")
            nc.sync.dma_start(t_sq[:], sqrow[:])
            t_sb = pp.tile([128, 8], F32, tag="sb")
            nc.sync.dma_start(t_sb[:], sqbias[:])
            t_sb2 = pp.tile([128, 8], F32, tag="sb2")
            nc.sync.dma_start(t_sb2[:], sqbias2[:])
            t_o1 = pp.tile([2, 128], BF16, tag="o1")
            nc.sync.dma_start(t_o1[:], ones1[:])

            t_accs = pp.tile([128, 32], F32, tag="accs")

            # col tiles per m: [lo, lo+2048), [lo+2048, lo+4096), [lo+4096, lo+4608)
            # First 512 cols of tile 0 = diag block: exact sqrt/hinge/w^2.
            # Everything else: zero-screen  sum(min(d2+eps,1)-1)  (== 0 iff
            # no pair violates the margin; host falls back if nonzero).
            for m in range(8):
                base = 0 if m < 4 else 4096
                for t, (c0, cw) in enumerate([(0, 2048), (2048, 2048),
                                              (4096, 512)]):
                    lo = base + c0
                    pt = psp.tile([128, 2048], F32, tag="pt")
                    nmm = cw // 512
                    for j in range(nmm):
                        nc.tensor.matmul(
                            pt[:, 512 * j:512 * (j + 1)],
                            t_lh[:, 128 * m:128 * (m + 1)],
                            t_ctr[:, lo + 512 * j:lo + 512 * (j + 1)],
                            start=True, stop=False)
                    for j in range(nmm):
                        nc.tensor.matmul(
                            pt[:, 512 * j:512 * (j + 1)],
                            t_o1[:2, :],
                            t_sq[:2, lo + 512 * j:lo + 512 * (j + 1)],
                            start=False, stop=True)
                    if t == 0:
                        # diag block: exact path on first 512 cols
                        th = wp.tile([128, 512], F32, tag="th")
                        nc.scalar.activation(th[:], pt[:, :512], AF.Sqrt,
                                             bias=t_sb[:, m:m + 1], scale=1.0)
                        tw = wp.tile([128, 512], F32, tag="tw")
                        nc.vector.tensor_scalar(tw[:], th[:], 1.0, 1.0,
                                                op0=OP.min, op1=OP.subtract)
                        tw2 = wp.tile([128, 512], F32, tag="tw2")
                        nc.vector.scalar_tensor_tensor(
                            tw2[:], tw[:], 0.0, tw[:],
                            op0=OP.bypass, op1=OP.mult,
                            accum_out=t_accs[:, 4 * m:4 * m + 1])
                        ts = wp.tile([128, 1536], F32, tag="ts0")
                        nc.vector.tensor_scalar(
                            ts[:], pt[:, 512:2048], t_sb2[:, m:m + 1], 1.0,
                            op0=OP.add, op1=OP.min,
                            accum_out=t_accs[:, 4 * m + 1:4 * m + 2])
                    elif t == 1:
                        ts = wp.tile([128, 2048], F32, tag="ts1")
                        nc.vector.tensor_scalar(
                            ts[:], pt[:, :2048], t_sb2[:, m:m + 1], 1.0,
                            op0=OP.add, op1=OP.min,
                            accum_out=t_accs[:, 4 * m + 2:4 * m + 3])
                    else:
                        ts = wp.tile([128, 512], F32, tag="ts2")
                        nc.vector.tensor_scalar(
                            ts[:], pt[:, :512], t_sb2[:, m:m + 1], 1.0,
                            op0=OP.add, op1=OP.min,
                            accum_out=t_accs[:, 4 * m + 3:4 * m + 4])
            nc.sync.dma_start(accs[:], t_accs[:])
    nc.compile()
    return nc


def _get(name, builder):
    if name not in _cache:
        _cache[name] = builder()
    return _cache[name]


def kernel(path_fea):
    fea = np.asarray(path_fea, dtype=np.float32).reshape(B, D)

    trace = bool(int(__import__("os").environ.get("KERNEL_TRACE", "0")))
    runkw = {}
    if trace:
        import trace_shim
        trace_shim.install()
        runkw = dict(trace=True)

    # ---------------- launch 1 ----------------
    nc1 = _get("l1", _build_launch1)
    xbf = fea.astype(ml_dtypes.bfloat16)
    sel = np.zeros((128, 8), np.float32)
    for s in range(128):
        sel[s, s // 16] = 1.0 / 16.0
    sel = sel.astype(ml_dtypes.bfloat16)
    in1 = []
    for c in range(N_CORES):
        sh = xbf[SL * c:SL * (c + 1)].reshape(NT, 128, D).transpose(1, 0, 2)
        in1.append({"xp": np.ascontiguousarray(sh.reshape(128, SL)),
                    "sel": sel})
    r1 = run_bass_kernel_spmd(nc1, in1, core_ids=list(range(N_CORES)), **runkw)
    if trace and r1.exec_time_ns is not None:
        print(f"[launch1] HW exec time: {r1.exec_time_ns} ns")
        _last_traces["launch1"] = r1

    # ---------------- host gather ----------------
    centers = np.empty((G, D), ml_dtypes.bfloat16)
    ipart_sum = 0.0
    for c in range(N_CORES):
        cp = r1.results[c]["cpack"].reshape(8, NT, D)      # slot b d
        centers[GL * c:GL * (c + 1)] = cp.transpose(1, 0, 2).reshape(GL, D)
        ipart_sum += float(r1.results[c]["ipart"].astype(np.float64).sum())

    cf = centers.astype(np.float32)
    sq = (cf ** 2).sum(1)                                  # [G] f32
    hi = sq.astype(ml_dtypes.bfloat16)
    lo = (sq - hi.astype(np.float32)).astype(ml_dtypes.bfloat16)
    ctrT = np.ascontiguousarray(centers.T)                 # [128, G] bf16
    ones1 = np.ones((2, 128), np.float32).astype(ml_dtypes.bfloat16)

    in2 = []
    rowsets = []
    for c in range(N_CORES):
        idx = (np.arange(EXT) + CW * c) % G
        ctr_ext = np.ascontiguousarray(ctrT[:, idx])
        sqrow = np.ascontiguousarray(np.stack([hi[idx], lo[idx]]))
        rows = np.concatenate([np.arange(CW * c, CW * (c + 1)),
                               np.arange(CW * (c + 8), CW * (c + 9))])
        rowsets.append(rows)
        lhT = np.ascontiguousarray(
            (cf[rows].T * np.float32(-2.0)).astype(ml_dtypes.bfloat16))
        sqb = np.ascontiguousarray(
            (sq[rows] + np.float32(EPS)).reshape(8, 128).T)
        sqb2 = np.ascontiguousarray(sq[rows].reshape(8, 128).T)
        in2.append({"ctr": ctr_ext, "lh": lhT, "sqrow": sqrow,
                    "sqbias": sqb, "sqbias2": sqb2, "ones1": ones1})

    nc2 = _get("l2", _build_launch2)
    r2 = run_bass_kernel_spmd(nc2, in2, core_ids=list(range(N_CORES)), **runkw)
    if trace and r2.exec_time_ns is not None:
        print(f"[launch2] HW exec time: {r2.exec_time_ns} ns")
        _last_traces["launch2"] = r2

    # ---------------- host reduce ----------------
    # screen columns must equal their column counts (every min(d2,1)==1);
    # otherwise some pair violates the margin -> exact host fallback.
    S_diag = 0.0
    clean = True
    expect = {1: 1536.0, 2: 2048.0, 3: 512.0}
    for c in range(N_CORES):
        a = r2.results[c]["accs"].astype(np.float64)       # [128, 32]
        for m in range(8):
            S_diag += a[:, 4 * m].sum()
            for t, e in expect.items():
                if not np.all(a[:, 4 * m + t] == e):
                    clean = False

    if not clean:
        # margin violations exist: compute inter exactly on host (slow path,
        # never taken for margin-respecting data)
        cd = cf.astype(np.float64)
        sqd = (cd ** 2).sum(1)
        inter_sum = 0.0
        for i0 in range(0, G, 1024):
            blk = sqd[i0:i0 + 1024, None] + sqd[None, :] \
                - 2.0 * (cd[i0:i0 + 1024] @ cd.T)
            dmat = np.sqrt(np.maximum(blk, 0.0))
            h2 = np.maximum(1.0 - dmat, 0.0) ** 2
            iu = np.triu(np.ones((1024, G), dtype=bool), k=1 + i0)
            inter_sum += h2[iu].sum()
    else:
        # host model of the device's self-pair terms (d^2~0 -> t~sqrt(eps))
        S_self = 0.0
        f32 = np.float32
        for c in range(N_CORES):
            rows = rowsets[c]
            s = sq[rows]
            h = hi[rows].astype(np.float32)
            l = lo[rows].astype(np.float32)
            arg = (f32(-2.0) * s + (h + l)) + (s + f32(EPS))
            t = np.sqrt(np.maximum(arg, 0.0))
            w = np.minimum(t, 1.0) - 1.0
            S_self += float((w.astype(np.float64) ** 2).sum())
        inter_sum = (S_diag - S_self) / 2.0

    n_pairs = G * (G - 1) / 2.0
    inter = np.float32(inter_sum / n_pairs)
    intra = np.float32(ipart_sum / (G * P))
    return (inter, intra)


# revision 16
# speedup vs baseline: 1.0502x; 1.0502x over previous
"""LDA loss (inter/intra hinge) on 8 Trainium2 NeuronCores.

Strategy (data-parallel over B; G=B/16 centers; all-bf16 data path):

  Host staging: cast path_fea to bf16, rearrange each core's shard to
    p-major [128, b, d] so the device load is fully contiguous.

  Launch 1 (per core, 16384 samples):
    - centers via PE selector matmul ([8, 2048] psum per 16-tile chunk)
    - ACT casts centers to bf16 (export + reuse)
    - DMA replicates center rows across partition groups (16x)
    - DVE diff = x - c, ACT square, DVE segmented reduce -> d2 [128,128]
    - tiny f32 hinge tail -> ipart [128, 1]

  Host: gather centers (bf16), build per-core rotated center panels.

  Launch 2 (per core, cyclic-half of the GxG pairwise matrix):
    Uniform triangle: 16 row-chunks of 512; core c owns chunks c and c+8.
    Each row-chunk processes 9 column blocks (its own + next 8 mod 16)
    from a rotated+extended center panel [128, 8704]:
      psum = -2*C_loc^T C_ext + (ones^T [hi;lo]) (K=2 matmul adds ||c_j||^2)
      ACT: t = sqrt(psum + ||c_i||^2 + eps)  (bias per partition)
      DVE: w = min(t,1)-1 ; sum w^2 via scalar_tensor_tensor accum
    Separate accumulators for diag block (self pairs + double-counted),
    middle blocks (counted once), and far block (double-counted).
  Host: inter = (S_mid + (S_diag - S_self)/2 + S_far/2) / n_pairs.

Exact-zero property: every pair with d >= 1 yields min(t,1)-1 = 0
exactly, so the all-zero inter loss is reproduced up to the tiny
host-side self-pair correction (~1e-7 absolute).
"""
import sys

if "/opt/trn_rl_repo" not in sys.path:
    sys.path.insert(0, "/opt/trn_rl_repo")

import numpy as np
import ml_dtypes

import concourse.bacc as bacc
import concourse.tile as tile
from concourse import mybir
from concourse.bass_utils import run_bass_kernel_spmd

N_CORES = 8
B, D, P = 131072, 128, 16
G = B // P                 # 8192 centers
GL = G // N_CORES          # 1024 local centers (rows) per core
SL = B // N_CORES          # 16384 local samples
NT = SL // 128             # 128 sample tiles / core
CW = 512                   # row-chunk width (16 chunks globally)
EXT = 17 * CW              # 8704 extended column panel
EPS = 1e-3
MI = 0.1

F32 = mybir.dt.float32
BF16 = mybir.dt.bfloat16
AF = mybir.ActivationFunctionType
OP = mybir.AluOpType
AX = mybir.AxisListType

_cache = {}
_last_traces = {}


def _build_launch1():
    nc = bacc.Bacc("TRN2", target_bir_lowering=False, debug=False,
                   num_devices=N_CORES)
    xp = nc.dram_tensor("xp", [128, SL], BF16, kind="ExternalInput").ap()
    sel = nc.dram_tensor("sel", [128, 8], BF16, kind="ExternalInput").ap()
    ident = nc.dram_tensor("ident", [128, 128], BF16, kind="ExternalInput").ap()
    negE = nc.dram_tensor("negE", [8, 128], BF16, kind="ExternalInput").ap()
    cpack = nc.dram_tensor("cpack", [8, SL], BF16, kind="ExternalOutput").ap()
    ipart = nc.dram_tensor("ipart", [128, 1], F32, kind="ExternalOutput").ap()

    with tile.TileContext(nc) as tc:
        with (
            tc.tile_pool(name="persist", bufs=1) as pp,
            tc.tile_pool(name="work", bufs=3) as wp,
            tc.tile_pool(name="ps1", bufs=2, space="PSUM") as psp,
        ):
            t_sel = pp.tile([128, 8], BF16, tag="sel")
            nc.sync.dma_start(t_sel[:], sel[:])
            t_id = pp.tile([128, 128], BF16, tag="id")
            nc.sync.dma_start(t_id[:], ident[:])
            t_ne = pp.tile([8, 128], BF16, tag="ne")
            nc.sync.dma_start(t_ne[:], negE[:])
            t_xp = pp.tile([128, SL], BF16, tag="xp")
            for k in range(4):
                nc.sync.dma_start(t_xp[:, 4096 * k:4096 * (k + 1)],
                                  xp[:, 4096 * k:4096 * (k + 1)])
            t_d2 = pp.tile([128, NT], F32, tag="d2")
            t_cseg = pp.tile([8, SL], BF16, tag="cseg")

            # centers: selector matmul per 2048-col chunk, cast to bf16
            for k in range(8):
                sl2 = slice(2048 * k, 2048 * (k + 1))
                cps = psp.tile([128, 2048], F32, tag="ps")
                for j in range(4):
                    nc.tensor.matmul(
                        cps[:8, 512 * j:512 * (j + 1)], t_sel[:, :],
                        t_xp[:, 2048 * k + 512 * j:2048 * k + 512 * (j + 1)],
                        start=True, stop=True)
                nc.vector.tensor_scalar(t_cseg[:, sl2], cps[:8, :], 0.0, None,
                                        op0=OP.add)
            nc.sync.dma_start(cpack[:, :], t_cseg[:, :])

            # diff = I*x - E*c in PSUM, square (ACT/DVE split), reduce over d
            for k in range(8):
                sl2 = slice(2048 * k, 2048 * (k + 1))
                dps = psp.tile([128, 2048], F32, tag="ps")
                for j in range(4):
                    nc.tensor.matmul(
                        dps[:, 512 * j:512 * (j + 1)], t_id[:, :],
                        t_xp[:, 2048 * k + 512 * j:2048 * k + 512 * (j + 1)],
                        start=True, stop=False)
                for j in range(4):
                    nc.tensor.matmul(
                        dps[:, 512 * j:512 * (j + 1)], t_ne[:, :],
                        t_cseg[:8, 2048 * k + 512 * j:2048 * k + 512 * (j + 1)],
                        start=False, stop=True)
                dsq = wp.tile([128, 2048], F32, tag="dsq")
                nc.scalar.activation(dsq[:], dps[:], AF.Square)
                nc.vector.tensor_reduce(
                    t_d2[:, 16 * k:16 * (k + 1)],
                    dsq[:].rearrange("p (b d) -> p b d", d=128),
                    axis=AX.X, op=OP.add)

            t_di = pp.tile([128, NT], F32, tag="di")
            nc.scalar.activation(t_di[:], t_d2[:], AF.Sqrt)
            t_w = pp.tile([128, NT], F32, tag="w")
            nc.vector.tensor_scalar(t_w[:], t_di[:], MI, 0.0,
                                    op0=OP.subtract, op1=OP.max)
            t_w2 = pp.tile([128, NT], F32, tag="w2")
            t_acc = pp.tile([128, 1], F32, tag="acc")
            nc.vector.scalar_tensor_tensor(t_w2[:], t_w[:], 0.0, t_w[:],
                                           op0=OP.bypass, op1=OP.mult,
                                           accum_out=t_acc[:])
            nc.sync.dma_start(ipart[:], t_acc[:])
    nc.compile()
    return nc


def _build_launch2():
    nc = bacc.Bacc("TRN2", target_bir_lowering=False, debug=False,
                   num_devices=N_CORES)
    ctr = nc.dram_tensor("ctr", [128, EXT], BF16, kind="ExternalInput").ap()
    lh = nc.dram_tensor("lh", [128, GL], BF16, kind="ExternalInput").ap()
    sqrow = nc.dram_tensor("sqrow", [2, EXT], BF16, kind="ExternalInput").ap()
    sqbias = nc.dram_tensor("sqbias", [128, 8], F32, kind="ExternalInput").ap()
    sqbias2 = nc.dram_tensor("sqbias2", [128, 8], F32,
                             kind="ExternalInput").ap()
    ones1 = nc.dram_tensor("ones1", [2, 128], BF16, kind="ExternalInput").ap()
    accs = nc.dram_tensor("accs", [128, 32], F32, kind="ExternalOutput").ap()

    with tile.TileContext(nc) as tc:
        with (
            tc.tile_pool(name="persist", bufs=1) as pp,
            tc.tile_pool(name="work", bufs=3) as wp,
            tc.tile_pool(name="ps", bufs=2, space="PSUM") as psp,
        ):
            t_lh = pp.tile([128, GL], BF16, tag="lh")
            nc.sync.dma_start(t_lh[:], lh[:])
            t_sq = pp.tile([2, EXT], BF16, tag="sq")
            nc.sync.dma_start(t_sq[:], sqrow[:])
            t_sb = pp.tile([128, 8], F32, tag="sb")
            nc.sync.dma_start(t_sb[:], sqbias[:])
            t_sb2 = pp.tile([128, 8], F32, tag="sb2")
            nc.sync.dma_start(t_sb2[:], sqbias2[:])
            t_o1 = pp.tile([2, 128], BF16, tag="o1")
            nc.sync.dma_start(t_o1[:], ones1[:])
            t_ctr = pp.tile([128, EXT], BF16, tag="ctr")
            for k in range(4):
                nc.sync.dma_start(t_ctr[:, 2048 * k:2048 * (k + 1)],
                                  ctr[:, 2048 * k:2048 * (k + 1)])
            nc.sync.dma_start(t_ctr[:, 8192:EXT], ctr[:, 8192:EXT])

            t_accs = pp.tile([128, 32], F32, tag="accs")

            # PE warm-up: dense matmul burst on already-loaded lh while ctr
            # still streams in (HAM needs ~4us of sustained PE activity to
            # lift the 1.2GHz cold clock gate)
            pw = psp.tile([128, 2048], F32, tag="pt")
            for _ in range(12):
                nc.tensor.matmul(pw[:, :512], t_lh[:, :128], t_lh[:, :512],
                                 start=True, stop=True)

            # col tiles per m: [lo, lo+2048), [lo+2048, lo+4096), [lo+4096, lo+4608)
            # First 512 cols of tile 0 = diag block: exact sqrt/hinge/w^2.
            # Everything else: zero-screen  sum(min(d2+eps,1)-1)  (== 0 iff
            # no pair violates the margin; host falls back if nonzero).
            for m in range(8):
                base = 0 if m < 4 else 4096
                for t, (c0, cw) in enumerate([(0, 2048), (2048, 2048),
                                              (4096, 512)]):
                    lo = base + c0
                    pt = psp.tile([128, 2048], F32, tag="pt")
                    nmm = cw // 512
                    for j in range(nmm):
                        nc.tensor.matmul(
                            pt[:, 512 * j:512 * (j + 1)],
                            t_lh[:, 128 * m:128 * (m + 1)],
                            t_ctr[:, lo + 512 * j:lo + 512 * (j + 1)],
                            start=True, stop=False)
                    for j in range(nmm):
                        nc.tensor.matmul(
                            pt[:, 512 * j:512 * (j + 1)],
                            t_o1[:2, :],
                            t_sq[:2, lo + 512 * j:lo + 512 * (j + 1)],
                            start=False, stop=True)
                    if t == 0:
                        # diag block: exact path on first 512 cols
                        th = wp.tile([128, 512], F32, tag="th")
                        nc.scalar.activation(th[:], pt[:, :512], AF.Sqrt,
                                             bias=t_sb[:, m:m + 1], scale=1.0)
                        tw = wp.tile([128, 512], F32, tag="tw")
                        nc.vector.tensor_scalar(tw[:], th[:], 1.0, 1.0,
                                                op0=OP.min, op1=OP.subtract)
                        tw2 = wp.tile([128, 512], F32, tag="tw2")
                        nc.vector.scalar_tensor_tensor(
                            tw2[:], tw[:], 0.0, tw[:],
                            op0=OP.bypass, op1=OP.mult,
                            accum_out=t_accs[:, 4 * m:4 * m + 1])
                        ts = wp.tile([128, 1536], F32, tag="ts0")
                        nc.vector.tensor_scalar(
                            ts[:], pt[:, 512:2048], t_sb2[:, m:m + 1], 1.0,
                            op0=OP.add, op1=OP.min,
                            accum_out=t_accs[:, 4 * m + 1:4 * m + 2])
                    elif t == 1:
                        ts = wp.tile([128, 2048], F32, tag="ts1")
                        nc.vector.tensor_scalar(
                            ts[:], pt[:, :2048], t_sb2[:, m:m + 1], 1.0,
                            op0=OP.add, op1=OP.min,
                            accum_out=t_accs[:, 4 * m + 2:4 * m + 3])
                    else:
                        ts = wp.tile([128, 512], F32, tag="ts2")
                        nc.vector.tensor_scalar(
                            ts[:], pt[:, :512], t_sb2[:, m:m + 1], 1.0,
                            op0=OP.add, op1=OP.min,
                            accum_out=t_accs[:, 4 * m + 3:4 * m + 4])
            nc.sync.dma_start(accs[:], t_accs[:])
    nc.compile()
    return nc


def _get(name, builder):
    if name not in _cache:
        _cache[name] = builder()
    return _cache[name]


def kernel(path_fea):
    fea = np.asarray(path_fea, dtype=np.float32).reshape(B, D)

    trace = bool(int(__import__("os").environ.get("KERNEL_TRACE", "0")))
    runkw = {}
    if trace:
        import trace_shim
        trace_shim.install()
        runkw = dict(trace=True)

    # ---------------- launch 1 ----------------
    nc1 = _get("l1", _build_launch1)
    xbf = fea.astype(ml_dtypes.bfloat16)
    sel = np.zeros((128, 8), np.float32)
    for s in range(128):
        sel[s, s // 16] = 1.0 / 16.0
    sel = sel.astype(ml_dtypes.bfloat16)
    ident = np.eye(128, dtype=np.float32).astype(ml_dtypes.bfloat16)
    negE = np.zeros((8, 128), np.float32)
    for s in range(128):
        negE[s // 16, s] = -1.0
    negE = negE.astype(ml_dtypes.bfloat16)
    in1 = []
    for c in range(N_CORES):
        sh = xbf[SL * c:SL * (c + 1)].reshape(NT, 128, D).transpose(1, 0, 2)
        in1.append({"xp": np.ascontiguousarray(sh.reshape(128, SL)),
                    "sel": sel, "ident": ident, "negE": negE})
    r1 = run_bass_kernel_spmd(nc1, in1, core_ids=list(range(N_CORES)), **runkw)
    if trace and r1.exec_time_ns is not None:
        print(f"[launch1] HW exec time: {r1.exec_time_ns} ns")
        _last_traces["launch1"] = r1

    # ---------------- host gather ----------------
    centers = np.empty((G, D), ml_dtypes.bfloat16)
    ipart_sum = 0.0
    for c in range(N_CORES):
        cp = r1.results[c]["cpack"].reshape(8, NT, D)      # slot b d
        centers[GL * c:GL * (c + 1)] = cp.transpose(1, 0, 2).reshape(GL, D)
        ipart_sum += float(r1.results[c]["ipart"].astype(np.float64).sum())

    cf = centers.astype(np.float32)
    sq = (cf ** 2).sum(1)                                  # [G] f32
    hi = sq.astype(ml_dtypes.bfloat16)
    lo = (sq - hi.astype(np.float32)).astype(ml_dtypes.bfloat16)
    ctrT = np.ascontiguousarray(centers.T)                 # [128, G] bf16
    ones1 = np.ones((2, 128), np.float32).astype(ml_dtypes.bfloat16)

    in2 = []
    rowsets = []
    for c in range(N_CORES):
        idx = (np.arange(EXT) + CW * c) % G
        ctr_ext = np.ascontiguousarray(ctrT[:, idx])
        sqrow = np.ascontiguousarray(np.stack([hi[idx], lo[idx]]))
        rows = np.concatenate([np.arange(CW * c, CW * (c + 1)),
                               np.arange(CW * (c + 8), CW * (c + 9))])
        rowsets.append(rows)
        lhT = np.ascontiguousarray(
            (cf[rows].T * np.float32(-2.0)).astype(ml_dtypes.bfloat16))
        sqb = np.ascontiguousarray(
            (sq[rows] + np.float32(EPS)).reshape(8, 128).T)
        sqb2 = np.ascontiguousarray(sq[rows].reshape(8, 128).T)
        in2.append({"ctr": ctr_ext, "lh": lhT, "sqrow": sqrow,
                    "sqbias": sqb, "sqbias2": sqb2, "ones1": ones1})

    nc2 = _get("l2", _build_launch2)
    r2 = run_bass_kernel_spmd(nc2, in2, core_ids=list(range(N_CORES)), **runkw)
    if trace and r2.exec_time_ns is not None:
        print(f"[launch2] HW exec time: {r2.exec_time_ns} ns")
        _last_traces["launch2"] = r2

    # ---------------- host reduce ----------------
    # screen columns must equal their column counts (every min(d2,1)==1);
    # otherwise some pair violates the margin -> exact host fallback.
    S_diag = 0.0
    clean = True
    expect = {1: 1536.0, 2: 2048.0, 3: 512.0}
    for c in range(N_CORES):
        a = r2.results[c]["accs"].astype(np.float64)       # [128, 32]
        for m in range(8):
            S_diag += a[:, 4 * m].sum()
            for t, e in expect.items():
                if not np.all(a[:, 4 * m + t] == e):
                    clean = False

    if not clean:
        # margin violations exist: compute inter exactly on host (slow path,
        # never taken for margin-respecting data)
        cd = cf.astype(np.float64)
        sqd = (cd ** 2).sum(1)
        inter_sum = 0.0
        for i0 in range(0, G, 1024):
            blk = sqd[i0:i0 + 1024, None] + sqd[None, :] \
                - 2.0 * (cd[i0:i0 + 1024] @ cd.T)
            dmat = np.sqrt(np.maximum(blk, 0.0))
            h2 = np.maximum(1.0 - dmat, 0.0) ** 2
            iu = np.triu(np.ones((1024, G), dtype=bool), k=1 + i0)
            inter_sum += h2[iu].sum()
    else:
        # host model of the device's self-pair terms (d^2~0 -> t~sqrt(eps))
        S_self = 0.0
        f32 = np.float32
        for c in range(N_CORES):
            rows = rowsets[c]
            s = sq[rows]
            h = hi[rows].astype(np.float32)
            l = lo[rows].astype(np.float32)
            arg = (f32(-2.0) * s + (h + l)) + (s + f32(EPS))
            t = np.sqrt(np.maximum(arg, 0.0))
            w = np.minimum(t, 1.0) - 1.0
            S_self += float((w.astype(np.float64) ** 2).sum())
        inter_sum = (S_diag - S_self) / 2.0

    n_pairs = G * (G - 1) / 2.0
    inter = np.float32(inter_sum / n_pairs)
    intra = np.float32(ipart_sum / (G * P))
    return (inter, intra)


# revision 24
# speedup vs baseline: 1.4954x; 1.4240x over previous
"""LDA loss (inter/intra hinge) on 8 Trainium2 NeuronCores.

Strategy (data-parallel over B; G=B/16 centers; all-bf16 data path):

  Host staging: cast path_fea to bf16, rearrange each core's shard to
    p-major [128, b, d] so the device load is fully contiguous.

  Launch 1 (per core, 16384 samples):
    - centers via PE selector matmul ([8, 2048] psum per 16-tile chunk)
    - ACT casts centers to bf16 (export + reuse)
    - DMA replicates center rows across partition groups (16x)
    - DVE diff = x - c, ACT square, DVE segmented reduce -> d2 [128,128]
    - tiny f32 hinge tail -> ipart [128, 1]

  Host: gather centers (bf16), build per-core rotated center panels.

  Launch 2 (per core, cyclic-half of the GxG pairwise matrix):
    Uniform triangle: 16 row-chunks of 512; core c owns chunks c and c+8.
    Each row-chunk processes 9 column blocks (its own + next 8 mod 16)
    from a rotated+extended center panel [128, 8704]:
      psum = -2*C_loc^T C_ext + (ones^T [hi;lo]) (K=2 matmul adds ||c_j||^2)
      ACT: t = sqrt(psum + ||c_i||^2 + eps)  (bias per partition)
      DVE: w = min(t,1)-1 ; sum w^2 via scalar_tensor_tensor accum
    Separate accumulators for diag block (self pairs + double-counted),
    middle blocks (counted once), and far block (double-counted).
  Host: inter = (S_mid + (S_diag - S_self)/2 + S_far/2) / n_pairs.

Exact-zero property: every pair with d >= 1 yields min(t,1)-1 = 0
exactly, so the all-zero inter loss is reproduced up to the tiny
host-side self-pair correction (~1e-7 absolute).
"""
import sys

if "/opt/trn_rl_repo" not in sys.path:
    sys.path.insert(0, "/opt/trn_rl_repo")

import numpy as np
import ml_dtypes

import concourse.bacc as bacc
import concourse.tile as tile
from concourse import mybir
from concourse.bass_utils import run_bass_kernel_spmd

N_CORES = 8
B, D, P = 131072, 128, 16
G = B // P                 # 8192 centers
GL = G // N_CORES          # 1024 local centers (rows) per core
SL = B // N_CORES          # 16384 local samples
NT = SL // 128             # 128 sample tiles / core
CW = 512                   # row-chunk width (16 chunks globally)
EXT = 17 * CW              # 8704 extended column panel
EPS = 1e-3
MI = 0.1

F32 = mybir.dt.float32
BF16 = mybir.dt.bfloat16
AF = mybir.ActivationFunctionType
OP = mybir.AluOpType
AX = mybir.AxisListType

_cache = {}
_last_traces = {}


def _build_launch1():
    nc = bacc.Bacc("TRN2", target_bir_lowering=False, debug=False,
                   num_devices=N_CORES)
    xp = nc.dram_tensor("xp", [128, SL], BF16, kind="ExternalInput").ap()
    sel = nc.dram_tensor("sel", [128, 128], BF16, kind="ExternalInput").ap()
    ident = nc.dram_tensor("ident", [128, 128], BF16, kind="ExternalInput").ap()
    negE = nc.dram_tensor("negE", [128, 128], BF16, kind="ExternalInput").ap()
    cpack = nc.dram_tensor("cpack", [8, SL], BF16, kind="ExternalOutput").ap()
    ipart = nc.dram_tensor("ipart", [128, 1], F32, kind="ExternalOutput").ap()

    with tile.TileContext(nc) as tc:
        with (
            tc.tile_pool(name="persist", bufs=1) as pp,
            tc.tile_pool(name="work", bufs=3) as wp,
            tc.tile_pool(name="ps1", bufs=2, space="PSUM") as psp,
        ):
            t_sel = pp.tile([128, 128], BF16, tag="sel")
            nc.sync.dma_start(t_sel[:], sel[:])
            t_id = pp.tile([128, 128], BF16, tag="id")
            nc.sync.dma_start(t_id[:], ident[:])
            t_ne = pp.tile([128, 128], BF16, tag="ne")
            nc.sync.dma_start(t_ne[:], negE[:])
            t_xp = pp.tile([128, SL], BF16, tag="xp")
            for k in range(4):
                nc.sync.dma_start(t_xp[:, 4096 * k:4096 * (k + 1)],
                                  xp[:, 4096 * k:4096 * (k + 1)])
            t_d2 = pp.tile([128, NT], F32, tag="d2")
            t_cseg = pp.tile([128, SL], BF16, tag="cseg")

            # PE warm-up on already-loaded weights
            pwu = psp.tile([128, 2048], F32, tag="ps")
            for _ in range(16):
                nc.tensor.matmul(pwu[:, :128], t_sel[:, :], t_id[:, :],
                                 start=True, stop=True)

            # centers: selector matmul per 2048-col chunk, cast to bf16
            # (sel cols replicate the 8 group slots, so every psum row
            # carries center data -> full PE array activity)
            for k in range(8):
                sl2 = slice(2048 * k, 2048 * (k + 1))
                cps = psp.tile([128, 2048], F32, tag="ps")
                for j in range(4):
                    nc.tensor.matmul(
                        cps[:, 512 * j:512 * (j + 1)], t_sel[:, :],
                        t_xp[:, 2048 * k + 512 * j:2048 * k + 512 * (j + 1)],
                        start=True, stop=True)
                if k % 2 == 0:
                    nc.scalar.copy(t_cseg[:, sl2], cps[:, :])
                else:
                    nc.vector.tensor_scalar(t_cseg[:, sl2], cps[:, :], 0.0,
                                            None, op0=OP.add)
            nc.sync.dma_start(cpack[:, :], t_cseg[:8, :])

            # diff = I*x - E*c in PSUM, square on ACT, reduce over d on DVE
            for k in range(8):
                sl2 = slice(2048 * k, 2048 * (k + 1))
                dps = psp.tile([128, 2048], F32, tag="ps")
                for j in range(4):
                    nc.tensor.matmul(
                        dps[:, 512 * j:512 * (j + 1)], t_id[:, :],
                        t_xp[:, 2048 * k + 512 * j:2048 * k + 512 * (j + 1)],
                        start=True, stop=False)
                for j in range(4):
                    nc.tensor.matmul(
                        dps[:, 512 * j:512 * (j + 1)], t_ne[:, :],
                        t_cseg[:, 2048 * k + 512 * j:2048 * k + 512 * (j + 1)],
                        start=False, stop=True)
                dsq = wp.tile([128, 2048], F32, tag="dsq")
                nc.scalar.activation(dsq[:], dps[:], AF.Square)
                nc.vector.tensor_reduce(
                    t_d2[:, 16 * k:16 * (k + 1)],
                    dsq[:].rearrange("p (b d) -> p b d", d=128),
                    axis=AX.X, op=OP.add)

            t_di = pp.tile([128, NT], F32, tag="di")
            nc.scalar.activation(t_di[:], t_d2[:], AF.Sqrt)
            t_w = pp.tile([128, NT], F32, tag="w")
            nc.vector.tensor_scalar(t_w[:], t_di[:], MI, 0.0,
                                    op0=OP.subtract, op1=OP.max)
            t_w2 = pp.tile([128, NT], F32, tag="w2")
            t_acc = pp.tile([128, 1], F32, tag="acc")
            nc.vector.scalar_tensor_tensor(t_w2[:], t_w[:], 0.0, t_w[:],
                                           op0=OP.bypass, op1=OP.mult,
                                           accum_out=t_acc[:])
            nc.sync.dma_start(ipart[:], t_acc[:])
    nc.compile()
    return nc


def _build_launch2():
    nc = bacc.Bacc("TRN2", target_bir_lowering=False, debug=False,
                   num_devices=N_CORES)
    ctr = nc.dram_tensor("ctr", [128, EXT], BF16, kind="ExternalInput").ap()
    lh = nc.dram_tensor("lh", [128, GL], BF16, kind="ExternalInput").ap()
    sqrow = nc.dram_tensor("sqrow", [128, EXT], BF16,
                           kind="ExternalInput").ap()
    sqbias = nc.dram_tensor("sqbias", [128, 8], F32, kind="ExternalInput").ap()
    sqbias2 = nc.dram_tensor("sqbias2", [128, 8], F32,
                             kind="ExternalInput").ap()
    sqbias3 = nc.dram_tensor("sqbias3", [128, 8], F32,
                             kind="ExternalInput").ap()
    ones1 = nc.dram_tensor("ones1", [128, 128], BF16,
                           kind="ExternalInput").ap()
    accs = nc.dram_tensor("accs", [128, 32], F32, kind="ExternalOutput").ap()

    with tile.TileContext(nc) as tc:
        with (
            tc.tile_pool(name="persist", bufs=1) as pp,
            tc.tile_pool(name="work", bufs=3) as wp,
            tc.tile_pool(name="ps", bufs=2, space="PSUM") as psp,
        ):
            t_lh = pp.tile([128, GL], BF16, tag="lh")
            nc.sync.dma_start(t_lh[:], lh[:])
            t_sb = pp.tile([128, 8], F32, tag="sb")
            nc.sync.dma_start(t_sb[:], sqbias[:])
            t_sb2 = pp.tile([128, 8], F32, tag="sb2")
            nc.sync.dma_start(t_sb2[:], sqbias2[:])
            t_sb3 = pp.tile([128, 8], F32, tag="sb3")
            nc.sync.dma_start(t_sb3[:], sqbias3[:])
            t_o1 = pp.tile([128, 128], BF16, tag="o1")
            nc.sync.dma_start(t_o1[:], ones1[:])
            t_ctr = pp.tile([128, EXT], BF16, tag="ctr")
            t_sq = pp.tile([128, EXT], BF16, tag="sq")
            for k in range(4):
                nc.sync.dma_start(t_ctr[:, 2048 * k:2048 * (k + 1)],
                                  ctr[:, 2048 * k:2048 * (k + 1)])
                nc.sync.dma_start(t_sq[:, 2048 * k:2048 * (k + 1)],
                                  sqrow[:, 2048 * k:2048 * (k + 1)])
            nc.sync.dma_start(t_ctr[:, 8192:EXT], ctr[:, 8192:EXT])
            nc.sync.dma_start(t_sq[:, 8192:EXT], sqrow[:, 8192:EXT])

            t_accs = pp.tile([128, 32], F32, tag="accs")

            # PE warm-up: dense matmul burst on already-loaded lh while ctr
            # still streams in (HAM needs ~4us of sustained PE activity to
            # lift the 1.2GHz cold clock gate)
            pw = psp.tile([128, 2048], F32, tag="pt")
            for _ in range(12):
                nc.tensor.matmul(pw[:, :512], t_lh[:, :128], t_lh[:, :512],
                                 start=True, stop=True)

            # col tiles per m: [lo, lo+2048), [lo+2048, lo+4096), [lo+4096, lo+4608)
            # First 512 cols of tile 0 = diag block: exact sqrt/hinge/w^2.
            # Everything else: zero-screen  sum(min(d2+eps,1)-1)  (== 0 iff
            # no pair violates the margin; host falls back if nonzero).
            for m in range(8):
                base = 0 if m < 4 else 4096
                for t, (c0, cw) in enumerate([(0, 2048), (2048, 2048),
                                              (4096, 512)]):
                    lo = base + c0
                    pt = psp.tile([128, 2048], F32, tag="pt")
                    nmm = cw // 512
                    for j in range(nmm):
                        nc.tensor.matmul(
                            pt[:, 512 * j:512 * (j + 1)],
                            t_lh[:, 128 * m:128 * (m + 1)],
                            t_ctr[:, lo + 512 * j:lo + 512 * (j + 1)],
                            start=True, stop=False)
                    for j in range(nmm):
                        nc.tensor.matmul(
                            pt[:, 512 * j:512 * (j + 1)],
                            t_o1[:, :],
                            t_sq[:, lo + 512 * j:lo + 512 * (j + 1)],
                            start=False, stop=True)
                    if t == 0:
                        # diag block: exact path on first 512 cols
                        th = wp.tile([128, 512], F32, tag="th")
                        nc.scalar.activation(th[:], pt[:, :512], AF.Sqrt,
                                             bias=t_sb[:, m:m + 1], scale=1.0)
                        tw = wp.tile([128, 512], F32, tag="tw")
                        nc.vector.tensor_scalar(tw[:], th[:], 1.0, 1.0,
                                                op0=OP.min, op1=OP.subtract)
                        tw2 = wp.tile([128, 512], F32, tag="tw2")
                        nc.vector.scalar_tensor_tensor(
                            tw2[:], tw[:], 0.0, tw[:],
                            op0=OP.bypass, op1=OP.mult,
                            accum_out=t_accs[:, 4 * m:4 * m + 1])
                        ts = wp.tile([128, 1536], F32, tag="ts0")
                        nc.vector.tensor_scalar(
                            ts[:], pt[:, 512:2048], t_sb2[:, m:m + 1], 1.0,
                            op0=OP.add, op1=OP.min,
                            accum_out=t_accs[:, 4 * m + 1:4 * m + 2])
                    elif t == 1:
                        # screen on ACT: relu(1 - d2) summed, 0 iff clean
                        ts = wp.tile([128, 2048], F32, tag="ts1")
                        nc.scalar.activation(
                            ts[:], pt[:, :2048], AF.Relu,
                            bias=t_sb3[:, m:m + 1], scale=-1.0,
                            accum_out=t_accs[:, 4 * m + 2:4 * m + 3])
                    else:
                        ts = wp.tile([128, 512], F32, tag="ts2")
                        nc.vector.tensor_scalar(
                            ts[:], pt[:, :512], t_sb2[:, m:m + 1], 1.0,
                            op0=OP.add, op1=OP.min,
                            accum_out=t_accs[:, 4 * m + 3:4 * m + 4])
            nc.sync.dma_start(accs[:], t_accs[:])
    nc.compile()
    return nc


def _get(name, builder):
    if name not in _cache:
        _cache[name] = builder()
    return _cache[name]


def kernel(path_fea):
    fea = np.asarray(path_fea, dtype=np.float32).reshape(B, D)

    trace = bool(int(__import__("os").environ.get("KERNEL_TRACE", "0")))
    runkw = {}
    if trace:
        import trace_shim
        trace_shim.install()
        runkw = dict(trace=True)

    # ---------------- launch 1 ----------------
    nc1 = _get("l1", _build_launch1)
    xbf = fea.astype(ml_dtypes.bfloat16)
    # sel: every output row m holds center (m % 8); negE: -1/16 weight on
    # the 16 replicated rows k with k % 8 == p // 16 -> exactly -c per row.
    sel = np.zeros((128, 128), np.float32)
    for s in range(128):
        for m in range(s // 16, 128, 8):
            sel[s, m] = 1.0 / 16.0
    sel = sel.astype(ml_dtypes.bfloat16)
    ident = np.eye(128, dtype=np.float32).astype(ml_dtypes.bfloat16)
    negE = np.zeros((128, 128), np.float32)
    for k in range(128):
        for p in range(128):
            if k % 8 == p // 16:
                negE[k, p] = -1.0 / 16.0
    negE = negE.astype(ml_dtypes.bfloat16)
    in1 = []
    for c in range(N_CORES):
        sh = xbf[SL * c:SL * (c + 1)].reshape(NT, 128, D).transpose(1, 0, 2)
        in1.append({"xp": np.ascontiguousarray(sh.reshape(128, SL)),
                    "sel": sel, "ident": ident, "negE": negE})
    r1 = run_bass_kernel_spmd(nc1, in1, core_ids=list(range(N_CORES)), **runkw)
    if trace and r1.exec_time_ns is not None:
        print(f"[launch1] HW exec time: {r1.exec_time_ns} ns")
        _last_traces["launch1"] = r1

    # ---------------- host gather ----------------
    centers = np.empty((G, D), ml_dtypes.bfloat16)
    ipart_sum = 0.0
    for c in range(N_CORES):
        cp = r1.results[c]["cpack"].reshape(8, NT, D)      # slot b d
        centers[GL * c:GL * (c + 1)] = cp.transpose(1, 0, 2).reshape(GL, D)
        ipart_sum += float(r1.results[c]["ipart"].astype(np.float64).sum())

    cf = centers.astype(np.float32)
    sq = (cf ** 2).sum(1)                                  # [G] f32
    hi = sq.astype(ml_dtypes.bfloat16)
    lo = (sq - hi.astype(np.float32)).astype(ml_dtypes.bfloat16)
    ctrT = np.ascontiguousarray(centers.T)                 # [128, G] bf16
    ones1 = np.ones((128, 128), np.float32).astype(ml_dtypes.bfloat16)

    in2 = []
    rowsets = []
    for c in range(N_CORES):
        idx = (np.arange(EXT) + CW * c) % G
        ctr_ext = np.ascontiguousarray(ctrT[:, idx])
        # sq panel: row0 hi, row1 lo, rows 2..127 alternate +hi/-hi so the
        # ones-weighted column sum stays hi+lo while every PE row toggles
        sqrow = np.empty((128, EXT), ml_dtypes.bfloat16)
        sqrow[0] = hi[idx]
        sqrow[1] = lo[idx]
        sqrow[2::2] = hi[idx]
        neg_hi = (-hi[idx].astype(np.float32)).astype(ml_dtypes.bfloat16)
        sqrow[3::2] = neg_hi
        sqrow = np.ascontiguousarray(sqrow)
        rows = np.concatenate([np.arange(CW * c, CW * (c + 1)),
                               np.arange(CW * (c + 8), CW * (c + 9))])
        rowsets.append(rows)
        lhT = np.ascontiguousarray(
            (cf[rows].T * np.float32(-2.0)).astype(ml_dtypes.bfloat16))
        sqb = np.ascontiguousarray(
            (sq[rows] + np.float32(EPS)).reshape(8, 128).T)
        sqb2 = np.ascontiguousarray(sq[rows].reshape(8, 128).T)
        sqb3 = np.ascontiguousarray(
            (np.float32(1.0) - sq[rows]).reshape(8, 128).T)
        in2.append({"ctr": ctr_ext, "lh": lhT, "sqrow": sqrow,
                    "sqbias": sqb, "sqbias2": sqb2, "sqbias3": sqb3,
                    "ones1": ones1})

    nc2 = _get("l2", _build_launch2)
    r2 = run_bass_kernel_spmd(nc2, in2, core_ids=list(range(N_CORES)), **runkw)
    if trace and r2.exec_time_ns is not None:
        print(f"[launch2] HW exec time: {r2.exec_time_ns} ns")
        _last_traces["launch2"] = r2

    # ---------------- host reduce ----------------
    # screen columns must equal their column counts (every min(d2,1)==1);
    # otherwise some pair violates the margin -> exact host fallback.
    S_diag = 0.0
    clean = True
    expect = {1: 1536.0, 2: 0.0, 3: 512.0}   # col2 is the ACT relu screen
    for c in range(N_CORES):
        a = r2.results[c]["accs"].astype(np.float64)       # [128, 32]
        for m in range(8):
            S_diag += a[:, 4 * m].sum()
            for t, e in expect.items():
                if not np.all(a[:, 4 * m + t] == e):
                    clean = False

    if not clean:
        # margin violations exist: compute inter exactly on host (slow path,
        # never taken for margin-respecting data)
        cd = cf.astype(np.float64)
        sqd = (cd ** 2).sum(1)
        inter_sum = 0.0
        for i0 in range(0, G, 1024):
            blk = sqd[i0:i0 + 1024, None] + sqd[None, :] \
                - 2.0 * (cd[i0:i0 + 1024] @ cd.T)
            dmat = np.sqrt(np.maximum(blk, 0.0))
            h2 = np.maximum(1.0 - dmat, 0.0) ** 2
            iu = np.triu(np.ones((1024, G), dtype=bool), k=1 + i0)
            inter_sum += h2[iu].sum()
    else:
        # host model of the device's self-pair terms (d^2~0 -> t~sqrt(eps))
        S_self = 0.0
        f32 = np.float32
        for c in range(N_CORES):
            rows = rowsets[c]
            s = sq[rows]
            h = hi[rows].astype(np.float32)
            l = lo[rows].astype(np.float32)
            arg = (f32(-2.0) * s + (h + l)) + (s + f32(EPS))
            t = np.sqrt(np.maximum(arg, 0.0))
            w = np.minimum(t, 1.0) - 1.0
            S_self += float((w.astype(np.float64) ** 2).sum())
        inter_sum = (S_diag - S_self) / 2.0

    n_pairs = G * (G - 1) / 2.0
    inter = np.float32(inter_sum / n_pairs)
    intra = np.float32(ipart_sum / (G * P))
    return (inter, intra)
